# revision 1
# baseline (speedup 1.0000x reference)
"""CrossNetMix (FuxiCTR MoE-routing) Trainium2 Bass kernel.

Math: the reference updates Xi = Xi + X0 * xw with xw of shape (B, 1), so
Xi is always a per-row scalar multiple of X0: Xi = c_b * X0[b].  With
precomputed per-row projections of X0 (g0 = X0@Wg^T, u0 = X0@U^T,
v0 = X0@V^T, p0 = sum_r u0*v0), each layer reduces to a tiny per-row scalar
recurrence:

    gate_logits = c * g0[l] + bg[l]
    xw          = c^2 * sum_e(p0[l] * softmax(gate_logits))
    c          += xw

so the whole network is ONE fused matmul X0 @ W_all^T (W_all = interleaved
U/V columns + gate columns, 3096 outputs/row) + an epilogue.  This is
exact (same flops as one reference layer-sweep; removes the sequential
layer dependency on the big matmuls).

Sharding: data-parallel over the batch dim across 8 NeuronCores; weights
replicated.  No collectives.

Per-core kernel (Bc = 2048 rows):
  - X0 shard is transposed host-side (layout prep) and DMA'd d-major so it
    can be the stationary matmul operand.
  - W is packed host-side into 13 groups of 256 columns: 12 UV groups
    [U(e)64 | V(e)64 | U(e+1)64 | V(e+1)64] (l = g//4) + 1 gate group (24
    cols, zero-padded).  Each group's matmul tile is (128k x 256n),
    accumulated over 16 k-tiles in PSUM.
  - Epilogue per (group, m-panel): DVE multiply U0*V0 (strided from PSUM)
    + segmented reduce over r -> p0; gate group copies g0.
  - Per m-panel: 3-layer scalar recurrence (DVE/ACT, per-partition scalar
    broadcasts) -> c, then out_panel = c * X0_panel.
"""

import os
import numpy as np

import concourse.bacc as bacc
import concourse.mybir as mybir
from concourse.tile import TileContext
from concourse.bass_utils import run_bass_kernel_spmd

# Problem constants (hardcoded per contest contract)
B, D, L, E, R = 16384, 2048, 3, 8, 64
N_CORES = 8
BC = B // N_CORES  # 2048 rows per core
P = 128
KT = D // P  # 16 contraction tiles
MT = BC // P  # 16 m-panels per core
GW = 512  # W-group width (columns)
N_UV = (L * E * 2 * R) // GW  # UV groups (6 at GW=512)
N_G = N_UV + 1  # +1 gate group
GATE_COLS = L * E  # 24
EPG = GW // 128  # experts per UV group (4 at GW=512)
GPL = (2 * E * R) // GW  # UV groups per layer (2 at GW=512)

_F32 = mybir.dt.float32


def build_nc(mm_dtype=mybir.dt.float32r, mt: int = MT, reps: int = 1,
             stage: int = 4):
    """Build the per-core Bass kernel. mt<MT builds a reduced-size kernel
    (for simulation); reps>1 wraps the body in a hardware loop (timing).
    stage: 1=matmul+copy only, 2=+ttr epilogue, 3=+recurrence, 4=full."""
    nc = bacc.Bacc("TRN2", target_bir_lowering=False, debug=False,
                   num_devices=N_CORES)
    bc = mt * P

    x0 = nc.dram_tensor("X0", [bc, D], _F32, kind="ExternalInput")
    xt = nc.dram_tensor("XT", [D, bc], _F32, kind="ExternalInput")
    w = nc.dram_tensor("W", [N_G, P, KT * GW], _F32, kind="ExternalInput")
    bgr = nc.dram_tensor("BG", [P, L * E], _F32, kind="ExternalInput")
    out = nc.dram_tensor("OUT", [bc, D], _F32, kind="ExternalOutput")

    with TileContext(nc) as tc:
        with (
            tc.tile_pool(name="xt_p", bufs=1) as xt_pool,
            tc.tile_pool(name="w_p", bufs=1) as w_pool,
            tc.tile_pool(name="pg_p", bufs=1) as pg_pool,
            tc.tile_pool(name="bg_p", bufs=1) as bg_pool,
            tc.tile_pool(name="prod_p", bufs=2) as prod_pool,
            tc.tile_pool(name="sm_p", bufs=2) as sm_pool,
            tc.tile_pool(name="xp_p", bufs=2) as xp_pool,
            tc.tile_pool(name="op_p", bufs=2) as op_pool,
            tc.tile_pool(name="ps_p", bufs=6, space="PSUM") as ps_pool,
        ):
            # --- persistent tiles ---
            xt_sb = xt_pool.tile([P, KT * bc], mm_dtype, tag="xt")
            bg_sb = bg_pool.tile([P, L * E], _F32, tag="bg")
            # per panel m: [p0 (L*E) | g0 (L*E)]
            pg_sb = pg_pool.tile([P, mt * 2 * GATE_COLS], _F32, tag="pg")
            c_sb = bg_pool.tile([P, mt], _F32, tag="c")

            nc.sync.dma_start(out=bg_sb[:], in_=bgr[:])

            # load X^T: 16 DMAs, one per k-slice (1 MB contiguous each).
            # gpsimd dma casts fp32 -> fp32r (rounding) when needed.
            dma_eng = nc.gpsimd if mm_dtype != _F32 else nc.sync
            for k in range(KT):
                dma_eng.dma_start(
                    out=xt_sb[:, k * bc : (k + 1) * bc],
                    in_=xt[k * P : (k + 1) * P, :],
                )

            def body(_iv=None):
                # init c = 1
                nc.vector.memset(c_sb[:], 1.0)
                for g in range(N_G):
                    is_gate = g == N_UV
                    ncols = GATE_COLS if is_gate else GW
                    w_sb = w_pool.tile([P, KT * GW], mm_dtype, tag="w")
                    dma_eng.dma_start(out=w_sb[:], in_=w[g])
                    for m in range(mt):
                        ps = ps_pool.tile([P, GW], _F32, tag="ps")
                        for k in range(KT):
                            nc.tensor.matmul(
                                ps[:, :ncols],
                                xt_sb[:, k * bc + m * P : k * bc + (m + 1) * P],
                                w_sb[:, k * GW : k * GW + ncols],
                                start=(k == 0),
                                stop=(k == KT - 1),
                            )
                        if is_gate:
                            nc.vector.tensor_copy(
                                pg_sb[:, m * 48 + 24 : m * 48 + 48],
                                ps[:, :GATE_COLS],
                            )
                        elif stage < 2:
                            nc.vector.tensor_copy(
                                pg_sb[:, m * 48 + g * 2 : m * 48 + g * 2 + 2],
                                ps[:, :2],
                            )
                        else:
                            l, ep = g // GPL, g % GPL
                            # PSUM -> SBUF (DVE reads only 1 PSUM src per op)
                            uvc = prod_pool.tile([P, GW], _F32, tag="uvc")
                            nc.vector.tensor_copy(uvc[:], ps[:])
                            uv4 = uvc[:].rearrange(
                                "p (e uv r) -> p e uv r", e=EPG, uv=2
                            )
                            prod = prod_pool.tile([P, EPG * R], _F32, tag="prod")
                            pv = prod[:].rearrange("p (e r) -> p e r", e=EPG)
                            nc.vector.tensor_tensor(
                                pv, uv4[:, :, 0, :], uv4[:, :, 1, :],
                                op=mybir.AluOpType.mult,
                            )
                            col = m * 48 + l * 8 + EPG * ep
                            nc.vector.reduce_sum(
                                pg_sb[:, col : col + EPG], pv,
                                axis=mybir.AxisListType.X,
                            )

                if stage < 3:
                    # just flush pg to OUT for inspection
                    op0 = op_pool.tile([P, mt * 2 * GATE_COLS], _F32, tag="op")
                    nc.vector.tensor_copy(op0[:], pg_sb[:])
                    nc.sync.dma_start(
                        out=out[0:P, : mt * 2 * GATE_COLS], in_=op0[:]
                    )
                    return
                # --- per-panel recurrence + output scale ---
                for m in range(mt):
                    c_m = c_sb[:, m : m + 1]
                    p0 = pg_sb[:, m * 48 : m * 48 + 24].rearrange(
                        "p (l e) -> p l e", l=L
                    )
                    g0 = pg_sb[:, m * 48 + 24 : m * 48 + 48].rearrange(
                        "p (l e) -> p l e", l=L
                    )
                    for l in range(L):
                        t = sm_pool.tile([P, E], _F32, tag="t")
                        et = sm_pool.tile([P, E], _F32, tag="et")
                        nmx = sm_pool.tile([P, 1], _F32, tag="nmx")
                        s1 = sm_pool.tile([P, 1], _F32, tag="s1")
                        s2 = sm_pool.tile([P, 1], _F32, tag="s2")
                        rcp = sm_pool.tile([P, 1], _F32, tag="rcp")
                        csq = sm_pool.tile([P, 1], _F32, tag="csq")
                        # t = c * g0[l] + bg[l]
                        nc.vector.scalar_tensor_tensor(
                            t[:], g0[:, l, :], c_m, bg_sb[:, l * E : (l + 1) * E],
                            op0=mybir.AluOpType.mult, op1=mybir.AluOpType.add,
                        )
                        # nmx = -max_e t
                        nc.vector.tensor_reduce(
                            nmx[:], t[:], axis=mybir.AxisListType.X,
                            op=mybir.AluOpType.max, negate=True,
                        )
                        # et = exp(t - max); s2 = sum_e et
                        nc.scalar.activation(
                            et[:], t[:], mybir.ActivationFunctionType.Exp,
                            bias=nmx[:], scale=1.0, accum_out=s2[:],
                        )
                        # w1 = p0[l] * et ; s1 = sum_e w1
                        nc.vector.scalar_tensor_tensor(
                            t[:], p0[:, l, :], 1.0, et[:],
                            op0=mybir.AluOpType.mult, op1=mybir.AluOpType.mult,
                            accum_out=s1[:],
                        )
                        nc.vector.reciprocal(rcp[:], s2[:])
                        # csq = c*c ; rcp = s1*rcp ; c += csq*rcp
                        nc.vector.tensor_tensor(
                            csq[:], c_m, c_m, op=mybir.AluOpType.mult
                        )
                        nc.vector.tensor_tensor(
                            rcp[:], s1[:], rcp[:], op=mybir.AluOpType.mult
                        )
                        nc.vector.scalar_tensor_tensor(
                            c_m, csq[:], rcp[:], c_m,
                            op0=mybir.AluOpType.mult, op1=mybir.AluOpType.add,
                        )
                    if stage >= 4:
                        # out panel = c * X0 panel (scaled in place)
                        xp = xp_pool.tile([P, D], _F32, tag="xp")
                        nc.sync.dma_start(
                            out=xp[:], in_=x0[m * P : (m + 1) * P, :]
                        )
                        nc.vector.tensor_scalar_mul(xp[:], xp[:], c_m)
                        nc.sync.dma_start(
                            out=out[m * P : (m + 1) * P, :], in_=xp[:]
                        )
                if stage == 3:
                    opc = op_pool.tile([P, mt], _F32, tag="op")
                    nc.vector.tensor_copy(opc[:], c_sb[:])
                    nc.sync.dma_start(out=out[0:P, :mt], in_=opc[:])

            if reps == 1:
                body()
            else:
                with tc.For_i(0, reps, 1) as iv:
                    body(iv)

    nc.compile()
    return nc


def pack_weights(U, V, Wg):
    """Host-side packing of U/V/Wg into (N_G, P, KT*GW) fp32."""
    # UV: (L, E, R, D) -> per (l, e): [U 64 | V 64] column block of width 128
    uv = np.stack([U, V], axis=2)  # (L, E, 2, R, D)
    uv = uv.reshape(L * E * 2 * R, D)  # rows = columns of the big matmul
    gate = np.concatenate(
        [Wg.reshape(L * E, D), np.zeros((GW - GATE_COLS, D), np.float32)], axis=0
    )
    allw = np.concatenate([uv, gate], axis=0)  # (N_G*GW, D)
    # -> (N_G, GW, KT, P) -> (N_G, P, KT, GW)
    allw = allw.reshape(N_G, GW, KT, P).transpose(0, 3, 2, 1)
    return np.ascontiguousarray(allw.reshape(N_G, P, KT * GW))


_CACHE = {}


def _get_runner(mm_dtype_name: str):
    key = mm_dtype_name
    if key not in _CACHE:
        _CACHE[key] = build_nc(getattr(mybir.dt, mm_dtype_name))
    return _CACHE[key]


def kernel(X0, U, V, Wg, bg):
    X0 = np.ascontiguousarray(np.asarray(X0, dtype=np.float32))
    Wpack = pack_weights(
        np.asarray(U, np.float32), np.asarray(V, np.float32),
        np.asarray(Wg, np.float32)
    )
    bg_rep = np.ascontiguousarray(
        np.broadcast_to(np.asarray(bg, np.float32).reshape(1, L * E), (P, L * E))
    )
    mm_dtype_name = os.environ.get("KERNEL_MM_DTYPE", "float32")
    nc = _get_runner(mm_dtype_name)

    in_maps = []
    for c in range(N_CORES):
        sh = X0[c * BC : (c + 1) * BC]
        in_maps.append(
            {
                "X0": sh,
                "XT": np.ascontiguousarray(sh.T),
                "W": Wpack,
                "BG": bg_rep,
            }
        )
    res = run_bass_kernel_spmd(nc, in_maps, list(range(N_CORES)))
    return np.concatenate([res.results[c]["OUT"] for c in range(N_CORES)], axis=0)



# revision 10
# speedup vs baseline: 2.2147x; 2.2147x over previous
"""CrossNetMix (FuxiCTR MoE-routing) Trainium2 Bass kernel.

Math: the reference updates Xi = Xi + X0 * xw with xw of shape (B, 1), so
Xi is always a per-row scalar multiple of X0: Xi = c_b * X0[b].  With
precomputed per-row projections of X0 (g0 = X0@Wg^T, u0 = X0@U^T,
v0 = X0@V^T, p0 = sum_r u0*v0), each layer reduces to a tiny per-row scalar
recurrence:

    gate_logits = c * g0[l] + bg[l]
    xw          = c^2 * sum_e(p0[l] * softmax(gate_logits))
    c          += xw

so the whole network is ONE fused matmul X0 @ W_all^T (W_all = interleaved
U/V columns + gate columns, 3096 outputs/row) + an epilogue.  This is
exact (same flops as one reference layer-sweep; removes the sequential
layer dependency on the big matmuls).

Sharding: data-parallel over the batch dim across 8 NeuronCores; weights
replicated.  No collectives.

Per-core kernel (Bc = 2048 rows):
  - X0 shard is transposed host-side (layout prep) and DMA'd d-major so it
    can be the stationary matmul operand.
  - W is packed host-side into 13 groups of 256 columns: 12 UV groups
    [U(e)64 | V(e)64 | U(e+1)64 | V(e+1)64] (l = g//4) + 1 gate group (24
    cols, zero-padded).  Each group's matmul tile is (128k x 256n),
    accumulated over 16 k-tiles in PSUM.
  - Epilogue per (group, m-panel): DVE multiply U0*V0 (strided from PSUM)
    + segmented reduce over r -> p0; gate group copies g0.
  - Per m-panel: 3-layer scalar recurrence (DVE/ACT, per-partition scalar
    broadcasts) -> c, then out_panel = c * X0_panel.
"""

import os
import numpy as np

import concourse.bacc as bacc
import concourse.mybir as mybir
from concourse.tile import TileContext
from concourse.bass_utils import run_bass_kernel_spmd

# Problem constants (hardcoded per contest contract)
B, D, L, E, R = 16384, 2048, 3, 8, 64
N_CORES = 8
BC = B // N_CORES  # 2048 rows per core
P = 128
KT = D // P  # 16 contraction tiles
MT = BC // P  # 16 m-panels per core
GW = 512  # W-group width (columns)
N_UV = (L * E * 2 * R) // GW  # UV groups (6 at GW=512)
N_G = N_UV + 1  # +1 gate group
GATE_COLS = L * E  # 24
EPG = GW // 128  # experts per UV group (4 at GW=512)
GPL = (2 * E * R) // GW  # UV groups per layer (2 at GW=512)

_F32 = mybir.dt.float32


def build_nc(mm_dtype=mybir.dt.float32r, mt: int = MT, reps: int = 1,
             stage: int = 4):
    """Build the per-core Bass kernel. mt<MT builds a reduced-size kernel
    (for simulation); reps>1 wraps the body in a hardware loop (timing).
    stage: 1=matmul+copy only, 2=+ttr epilogue, 3=+recurrence, 4=full."""
    nc = bacc.Bacc("TRN2", target_bir_lowering=False, debug=False,
                   num_devices=N_CORES)
    bc = mt * P

    x0 = nc.dram_tensor("X0", [bc, D], _F32, kind="ExternalInput")
    xt = nc.dram_tensor("XT", [D, bc], _F32, kind="ExternalInput")
    w = nc.dram_tensor("W", [N_G, P, KT * GW], _F32, kind="ExternalInput")
    bgr = nc.dram_tensor("BG", [P, L * E], _F32, kind="ExternalInput")
    out = nc.dram_tensor("OUT", [bc, D], _F32, kind="ExternalOutput")

    with TileContext(nc) as tc:
        with (
            tc.tile_pool(name="xt_p", bufs=1) as xt_pool,
            tc.tile_pool(name="w_p", bufs=1) as w_pool,
            tc.tile_pool(name="pg_p", bufs=1) as pg_pool,
            tc.tile_pool(name="bg_p", bufs=1) as bg_pool,
            tc.tile_pool(name="prod_p", bufs=2) as prod_pool,
            tc.tile_pool(name="sm_p", bufs=2) as sm_pool,
            tc.tile_pool(name="xp_p", bufs=2) as xp_pool,
            tc.tile_pool(name="op_p", bufs=2) as op_pool,
            tc.tile_pool(name="ps_p", bufs=6, space="PSUM") as ps_pool,
        ):
            # --- persistent tiles ---
            xt_sb = xt_pool.tile([P, KT * bc], mm_dtype, tag="xt")
            bg_sb = bg_pool.tile([P, L * E], _F32, tag="bg")
            # per panel m: [p0 (L*E) | g0 (L*E)]
            pg_sb = pg_pool.tile([P, mt * 2 * GATE_COLS], _F32, tag="pg")
            c_sb = bg_pool.tile([P, mt], _F32, tag="c")

            nc.sync.dma_start(out=bg_sb[:], in_=bgr[:])

            # load X^T: 16 DMAs, one per k-slice (1 MB contiguous each).
            # gpsimd dma casts fp32 -> fp32r (rounding) when needed.
            dma_eng = nc.gpsimd if mm_dtype != _F32 else nc.sync
            for k in range(KT):
                dma_eng.dma_start(
                    out=xt_sb[:, k * bc : (k + 1) * bc],
                    in_=xt[k * P : (k + 1) * P, :],
                )

            def body(_iv=None):
                # init c = 1
                nc.vector.memset(c_sb[:], 1.0)
                if stage == 5:
                    # X0 load / scale / store pipeline only
                    for m in range(mt):
                        xp = xp_pool.tile([P, D], _F32, tag="xp")
                        nc.sync.dma_start(
                            out=xp[:], in_=x0[m * P : (m + 1) * P, :]
                        )
                        nc.vector.tensor_scalar_mul(
                            xp[:], xp[:], c_sb[:, m : m + 1]
                        )
                        nc.sync.dma_start(
                            out=out[m * P : (m + 1) * P, :], in_=xp[:]
                        )
                    return
                for g in range(N_G):
                    is_gate = g == N_UV
                    ncols = GATE_COLS if is_gate else GW
                    w_sb = w_pool.tile([P, KT * GW], mm_dtype, tag="w")
                    dma_eng.dma_start(out=w_sb[:], in_=w[g])
                    if stage == 0:
                        # W DMA only: one consuming matmul per group
                        ps = ps_pool.tile([P, GW], _F32, tag="ps")
                        nc.tensor.matmul(
                            ps[:, :ncols], xt_sb[:, 0:P], w_sb[:, :ncols],
                            start=True, stop=True,
                        )
                        nc.vector.tensor_copy(
                            pg_sb[:, g * 2 : g * 2 + 2], ps[:, :2]
                        )
                        continue
                    for m in range(mt):
                        ps = ps_pool.tile([P, GW], _F32, tag="ps")
                        for k in range(KT):
                            nc.tensor.matmul(
                                ps[:, :ncols],
                                xt_sb[:, k * bc + m * P : k * bc + (m + 1) * P],
                                w_sb[:, k * GW : k * GW + ncols],
                                start=(k == 0),
                                stop=(k == KT - 1),
                            )
                        if is_gate:
                            nc.vector.tensor_copy(
                                pg_sb[:, m * 48 + 24 : m * 48 + 48],
                                ps[:, :GATE_COLS],
                            )
                        elif stage < 2:
                            nc.vector.tensor_copy(
                                pg_sb[:, m * 48 + g * 2 : m * 48 + g * 2 + 2],
                                ps[:, :2],
                            )
                        else:
                            l, ep = g // GPL, g % GPL
                            # PSUM -> SBUF (DVE reads only 1 PSUM src per op)
                            uvc = prod_pool.tile([P, GW], _F32, tag="uvc")
                            nc.vector.tensor_copy(uvc[:], ps[:])
                            uv4 = uvc[:].rearrange(
                                "p (e uv r) -> p e uv r", e=EPG, uv=2
                            )
                            prod = prod_pool.tile([P, EPG * R], _F32, tag="prod")
                            pv = prod[:].rearrange("p (e r) -> p e r", e=EPG)
                            nc.vector.tensor_tensor(
                                pv, uv4[:, :, 0, :], uv4[:, :, 1, :],
                                op=mybir.AluOpType.mult,
                            )
                            col = m * 48 + l * 8 + EPG * ep
                            nc.vector.reduce_sum(
                                pg_sb[:, col : col + EPG], pv,
                                axis=mybir.AxisListType.X,
                            )

                if stage == 0:
                    opz = op_pool.tile([P, 2 * N_G], _F32, tag="op")
                    nc.vector.tensor_copy(opz[:], pg_sb[:, : 2 * N_G])
                    nc.sync.dma_start(out=out[0:P, : 2 * N_G], in_=opz[:])
                    return
                if stage < 3:
                    # just flush pg to OUT for inspection
                    op0 = op_pool.tile([P, mt * 2 * GATE_COLS], _F32, tag="op")
                    nc.vector.tensor_copy(op0[:], pg_sb[:])
                    nc.sync.dma_start(
                        out=out[0:P, : mt * 2 * GATE_COLS], in_=op0[:]
                    )
                    return
                # --- per-panel recurrence + output scale ---
                for m in range(mt):
                    c_m = c_sb[:, m : m + 1]
                    p0 = pg_sb[:, m * 48 : m * 48 + 24].rearrange(
                        "p (l e) -> p l e", l=L
                    )
                    g0 = pg_sb[:, m * 48 + 24 : m * 48 + 48].rearrange(
                        "p (l e) -> p l e", l=L
                    )
                    for l in range(L):
                        t = sm_pool.tile([P, E], _F32, tag="t")
                        et = sm_pool.tile([P, E], _F32, tag="et")
                        nmx = sm_pool.tile([P, 1], _F32, tag="nmx")
                        s1 = sm_pool.tile([P, 1], _F32, tag="s1")
                        s2 = sm_pool.tile([P, 1], _F32, tag="s2")
                        rcp = sm_pool.tile([P, 1], _F32, tag="rcp")
                        csq = sm_pool.tile([P, 1], _F32, tag="csq")
                        # t = c * g0[l] + bg[l]
                        nc.vector.scalar_tensor_tensor(
                            t[:], g0[:, l, :], c_m, bg_sb[:, l * E : (l + 1) * E],
                            op0=mybir.AluOpType.mult, op1=mybir.AluOpType.add,
                        )
                        # nmx = -max_e t
                        nc.vector.tensor_reduce(
                            nmx[:], t[:], axis=mybir.AxisListType.X,
                            op=mybir.AluOpType.max, negate=True,
                        )
                        # et = exp(t - max); s2 = sum_e et
                        nc.scalar.activation(
                            et[:], t[:], mybir.ActivationFunctionType.Exp,
                            bias=nmx[:], scale=1.0, accum_out=s2[:],
                        )
                        # w1 = p0[l] * et ; s1 = sum_e w1
                        nc.vector.scalar_tensor_tensor(
                            t[:], p0[:, l, :], 1.0, et[:],
                            op0=mybir.AluOpType.mult, op1=mybir.AluOpType.mult,
                            accum_out=s1[:],
                        )
                        nc.vector.reciprocal(rcp[:], s2[:])
                        # csq = c*c ; rcp = s1*rcp ; c += csq*rcp
                        nc.vector.tensor_tensor(
                            csq[:], c_m, c_m, op=mybir.AluOpType.mult
                        )
                        nc.vector.tensor_tensor(
                            rcp[:], s1[:], rcp[:], op=mybir.AluOpType.mult
                        )
                        nc.vector.scalar_tensor_tensor(
                            c_m, csq[:], rcp[:], c_m,
                            op0=mybir.AluOpType.mult, op1=mybir.AluOpType.add,
                        )
                    if stage >= 4:
                        # out panel = c * X0 panel (scaled in place)
                        xp = xp_pool.tile([P, D], _F32, tag="xp")
                        nc.sync.dma_start(
                            out=xp[:], in_=x0[m * P : (m + 1) * P, :]
                        )
                        nc.vector.tensor_scalar_mul(xp[:], xp[:], c_m)
                        nc.sync.dma_start(
                            out=out[m * P : (m + 1) * P, :], in_=xp[:]
                        )
                if stage == 3:
                    opc = op_pool.tile([P, mt], _F32, tag="op")
                    nc.vector.tensor_copy(opc[:], c_sb[:])
                    nc.sync.dma_start(out=out[0:P, :mt], in_=opc[:])

            if reps == 1:
                body()
            else:
                with tc.For_i(0, reps, 1) as iv:
                    body(iv)

    nc.compile()
    return nc


def build_nc2(mm_dtype=mybir.dt.float16, mt: int = MT, reps: int = 1,
              stage: int = 4):
    """v2: weights preloaded to SBUF in mm_dtype (fits at <=2B), m-outer loop
    so each panel's epilogue/recurrence/scale/store pipelines behind the next
    panel's matmuls.  Gate group computed first within each panel."""
    nc = bacc.Bacc("TRN2", target_bir_lowering=False, debug=False,
                   num_devices=N_CORES)
    bc = mt * P

    x0 = nc.dram_tensor("X0", [bc, D], _F32, kind="ExternalInput")
    xt = nc.dram_tensor("XT", [D, bc], _F32, kind="ExternalInput")
    wuv = nc.dram_tensor("W", [N_UV, P, KT * GW], _F32, kind="ExternalInput")
    wgt = nc.dram_tensor("WG", [P, KT * GATE_COLS], _F32, kind="ExternalInput")
    bgr = nc.dram_tensor("BG", [P, L * E], _F32, kind="ExternalInput")
    out = nc.dram_tensor("OUT", [bc, D], _F32, kind="ExternalOutput")

    with TileContext(nc) as tc:
        with (
            tc.tile_pool(name="xt_p", bufs=1) as xt_pool,
            tc.tile_pool(name="w_p", bufs=1) as w_pool,
            tc.tile_pool(name="pg_p", bufs=1) as pg_pool,
            tc.tile_pool(name="bg_p", bufs=1) as bg_pool,
            tc.tile_pool(name="uvc_p", bufs=2) as uvc_pool,
            tc.tile_pool(name="prod_p", bufs=2) as prod_pool,
            tc.tile_pool(name="sm_p", bufs=2) as sm_pool,
            tc.tile_pool(name="xp_p", bufs=3) as xp_pool,
            tc.tile_pool(name="ps_p", bufs=8, space="PSUM") as ps_pool,
        ):
            # --- persistent tiles ---
            xt_sb = xt_pool.tile([P, KT * bc], mm_dtype, tag="xt")
            w_sb = w_pool.tile([P, N_UV * KT * GW], mm_dtype, tag="w")
            wg_sb = w_pool.tile([P, KT * GATE_COLS], mm_dtype, tag="wg")
            bg_sb = bg_pool.tile([P, L * E], _F32, tag="bg")
            pg_sb = pg_pool.tile([P, mt * 2 * GATE_COLS], _F32, tag="pg")
            c_sb = bg_pool.tile([P, mt], _F32, tag="c")

            # --- preamble: load + cast all weights and X^T (outside body) ---
            nc.sync.dma_start(out=bg_sb[:], in_=bgr[:])
            nc.gpsimd.dma_start(out=wg_sb[:], in_=wgt[:])
            for g in range(N_UV):
                nc.gpsimd.dma_start(
                    out=w_sb[:, g * KT * GW : (g + 1) * KT * GW], in_=wuv[g]
                )
            for k in range(KT):
                nc.gpsimd.dma_start(
                    out=xt_sb[:, k * bc : (k + 1) * bc],
                    in_=xt[k * P : (k + 1) * P, :],
                )

            def panel_recurrence(m):
                c_m = c_sb[:, m : m + 1]
                p0 = pg_sb[:, m * 48 : m * 48 + 24].rearrange(
                    "p (l e) -> p l e", l=L
                )
                g0 = pg_sb[:, m * 48 + 24 : m * 48 + 48].rearrange(
                    "p (l e) -> p l e", l=L
                )
                for l in range(L):
                    t = sm_pool.tile([P, E], _F32, tag="t")
                    et = sm_pool.tile([P, E], _F32, tag="et")
                    nmx = sm_pool.tile([P, 1], _F32, tag="nmx")
                    s1 = sm_pool.tile([P, 1], _F32, tag="s1")
                    s2 = sm_pool.tile([P, 1], _F32, tag="s2")
                    rcp = sm_pool.tile([P, 1], _F32, tag="rcp")
                    csq = sm_pool.tile([P, 1], _F32, tag="csq")
                    nc.vector.scalar_tensor_tensor(
                        t[:], g0[:, l, :], c_m, bg_sb[:, l * E : (l + 1) * E],
                        op0=mybir.AluOpType.mult, op1=mybir.AluOpType.add,
                    )
                    nc.vector.tensor_reduce(
                        nmx[:], t[:], axis=mybir.AxisListType.X,
                        op=mybir.AluOpType.max, negate=True,
                    )
                    nc.scalar.activation(
                        et[:], t[:], mybir.ActivationFunctionType.Exp,
                        bias=nmx[:], scale=1.0, accum_out=s2[:],
                    )
                    nc.vector.scalar_tensor_tensor(
                        t[:], p0[:, l, :], 1.0, et[:],
                        op0=mybir.AluOpType.mult, op1=mybir.AluOpType.mult,
                        accum_out=s1[:],
                    )
                    nc.vector.reciprocal(rcp[:], s2[:])
                    nc.vector.tensor_tensor(
                        csq[:], c_m, c_m, op=mybir.AluOpType.mult
                    )
                    nc.vector.tensor_tensor(
                        rcp[:], s1[:], rcp[:], op=mybir.AluOpType.mult
                    )
                    nc.vector.scalar_tensor_tensor(
                        c_m, csq[:], rcp[:], c_m,
                        op0=mybir.AluOpType.mult, op1=mybir.AluOpType.add,
                    )

            def body(_iv=None):
                nc.vector.memset(c_sb[:], 1.0)
                for m in range(mt):
                    # prefetch X0 panel for the final scale (vector DGE queue)
                    xp = xp_pool.tile([P, D], _F32, tag="xp")
                    nc.scalar.dma_start(out=xp[:], in_=x0[m * P : (m + 1) * P, :])
                    # gate group first: g0 for this panel
                    psg = ps_pool.tile([P, GW], _F32, tag="ps")
                    for k in range(KT):
                        nc.tensor.matmul(
                            psg[:, :GATE_COLS],
                            xt_sb[:, k * bc + m * P : k * bc + (m + 1) * P],
                            wg_sb[:, k * GATE_COLS : (k + 1) * GATE_COLS],
                            start=(k == 0),
                            stop=(k == KT - 1),
                        )
                    nc.scalar.copy(
                        pg_sb[:, m * 48 + 24 : m * 48 + 48], psg[:, :GATE_COLS]
                    )
                    for g in range(N_UV):
                        ps = ps_pool.tile([P, GW], _F32, tag="ps")
                        for k in range(KT):
                            nc.tensor.matmul(
                                ps[:],
                                xt_sb[:, k * bc + m * P : k * bc + (m + 1) * P],
                                w_sb[:, (g * KT + k) * GW : (g * KT + k + 1) * GW],
                                start=(k == 0),
                                stop=(k == KT - 1),
                            )
                        if stage < 2:
                            nc.vector.tensor_copy(
                                pg_sb[:, m * 48 + g * 2 : m * 48 + g * 2 + 2],
                                ps[:, :2],
                            )
                            continue
                        # PSUM -> SBUF on ACT; product+reduce on DVE
                        uvc = uvc_pool.tile([P, GW], _F32, tag="uvc")
                        nc.scalar.copy(uvc[:], ps[:])
                        uv4 = uvc[:].rearrange(
                            "p (e uv r) -> p e uv r", e=EPG, uv=2
                        )
                        prod = prod_pool.tile([P, EPG * R], _F32, tag="prod")
                        pv = prod[:].rearrange("p (e r) -> p e r", e=EPG)
                        nc.vector.tensor_tensor(
                            pv, uv4[:, :, 0, :], uv4[:, :, 1, :],
                            op=mybir.AluOpType.mult,
                        )
                        col = m * 48 + (g // GPL) * 8 + EPG * (g % GPL)
                        nc.vector.reduce_sum(
                            pg_sb[:, col : col + EPG], pv,
                            axis=mybir.AxisListType.X,
                        )
                    # recurrence + output scale + store for this panel
                    if stage >= 3:
                        panel_recurrence(m)
                    nc.vector.tensor_scalar_mul(xp[:], xp[:], c_sb[:, m : m + 1])
                    nc.sync.dma_start(
                        out=out[m * P : (m + 1) * P, :], in_=xp[:]
                    )

            if reps == 1:
                body()
            else:
                with tc.For_i(0, reps, 1) as iv:
                    body(iv)

    nc.compile()
    return nc


def pack_weights2(U, V, Wg):
    """Host-side packing for v2: UV groups (N_UV, P, KT*GW) + compact gate
    (P, KT*GATE_COLS), both fp32 (cast to mm dtype by the load DMA)."""
    uv = np.stack([U, V], axis=2).reshape(L * E * 2 * R, D)
    uv = uv.reshape(N_UV, GW, KT, P).transpose(0, 3, 2, 1)
    wuv = np.ascontiguousarray(uv.reshape(N_UV, P, KT * GW))
    gate = Wg.reshape(GATE_COLS, D).reshape(GATE_COLS, KT, P).transpose(2, 1, 0)
    wgt = np.ascontiguousarray(gate.reshape(P, KT * GATE_COLS))
    return wuv, wgt


def pack_weights(U, V, Wg):
    """Host-side packing of U/V/Wg into (N_G, P, KT*GW) fp32."""
    # UV: (L, E, R, D) -> per (l, e): [U 64 | V 64] column block of width 128
    uv = np.stack([U, V], axis=2)  # (L, E, 2, R, D)
    uv = uv.reshape(L * E * 2 * R, D)  # rows = columns of the big matmul
    gate = np.concatenate(
        [Wg.reshape(L * E, D), np.zeros((GW - GATE_COLS, D), np.float32)], axis=0
    )
    allw = np.concatenate([uv, gate], axis=0)  # (N_G*GW, D)
    # -> (N_G, GW, KT, P) -> (N_G, P, KT, GW)
    allw = allw.reshape(N_G, GW, KT, P).transpose(0, 3, 2, 1)
    return np.ascontiguousarray(allw.reshape(N_G, P, KT * GW))


_CACHE = {}


def _get_runner(version: str, mm_dtype_name: str):
    key = (version, mm_dtype_name)
    if key not in _CACHE:
        builder = build_nc2 if version == "2" else build_nc
        _CACHE[key] = builder(getattr(mybir.dt, mm_dtype_name))
    return _CACHE[key]


def make_in_maps(version, X0, U, V, Wg, bg):
    X0 = np.ascontiguousarray(np.asarray(X0, dtype=np.float32))
    U = np.asarray(U, np.float32)
    V = np.asarray(V, np.float32)
    Wg = np.asarray(Wg, np.float32)
    bg_rep = np.ascontiguousarray(
        np.broadcast_to(np.asarray(bg, np.float32).reshape(1, L * E), (P, L * E))
    )
    if version == "2":
        wuv, wgt = pack_weights2(U, V, Wg)
    else:
        wall = pack_weights(U, V, Wg)
    in_maps = []
    for c in range(N_CORES):
        sh = X0[c * BC : (c + 1) * BC]
        m = {"X0": sh, "XT": np.ascontiguousarray(sh.T), "BG": bg_rep}
        if version == "2":
            m["W"], m["WG"] = wuv, wgt
        else:
            m["W"] = wall
        in_maps.append(m)
    return in_maps


def kernel(X0, U, V, Wg, bg):
    version = os.environ.get("KERNEL_V", "2")
    mm_dtype_name = os.environ.get(
        "KERNEL_MM_DTYPE", "float16" if version == "2" else "float32"
    )
    nc = _get_runner(version, mm_dtype_name)
    in_maps = make_in_maps(version, X0, U, V, Wg, bg)
    res = run_bass_kernel_spmd(nc, in_maps, list(range(N_CORES)))
    return np.concatenate([res.results[c]["OUT"] for c in range(N_CORES)], axis=0)

